# revision 7
# baseline (speedup 1.0000x reference)
"""GNN NodeModel kernel for 8 Trainium2 NeuronCores.

Computation (see module docstring in the harness reference):
    e_aggr = scatter_mean(edge_attr, edge_index[1], n_nodes)       # [N, 96]
    h      = concat([x, e_aggr, u[batch]], 1) @ W1 + b1            # [N, 256]
    h      = BatchNorm1d(h, training stats) ; relu                 # global mean/var!
    out    = h @ W2 + b2                                           # [N, 128]

Sharding: nodes are split into 8 contiguous ranges of 6250 (one per core);
edges are binned by *target* node so every scatter target is core-local
(equivalent to, and simpler than, the per-graph hint — edge targets here have
no graph locality anyway).  MLP weights are replicated.  BatchNorm statistics
are globally AllReduced across the 8 cores on-device.

Device algorithm per core:
  Phase A (segment-mean): edges arrive pre-sorted by target, padded per
    128-node block to a uniform tile count S.  For each 128-edge tile the DVE
    builds a "value-hot" matrix  vhot[e, n] = (iota[n] == rel[e]) * inv_cnt[e]
    (bf16) and the PE accumulates  e_aggrT[block] += edge_tile^T @ vhot  in
    PSUM.  Folding 1/cnt into the one-hot makes the matmul produce the mean
    directly; pad edges carry rel=255 which never matches iota -> zero rows.
  Phase B (MLP1): P^T[half] = sum_k W1[k,half]^T @ act^T[k] over k-chunks
    {x(128), u[batch](64), e_aggr(96)}, feature-major so no transposes are
    needed anywhere.  b1 is dropped: BatchNorm is exactly invariant to any
    constant per-feature shift, so this is mathematically exact for any b1.
  Phase C (BN stats): per-feature sum (DVE reduce) and sum-of-squares (ACT
    Square with accum_out), AllReduce [128,4] across cores, then
    scale = gamma/sqrt(var+eps), shift = beta - mean*scale.
  Phase D: one ACT op per half: relu(P^T * scale + shift)  (fused BN+ReLU).
  Phase E (MLP2): out^T = sum_j W2[j]^T @ hrelu^T[j] + b2 (K=1 ones matmul),
    written transposed; the host transposes back while unsharding.

Edge values ride in bf16 (their contribution to h is small and BatchNorm is
scale-invariant); all MLP math is fp32 (float32r PE fast path).
"""

import os
import sys

sys.path.insert(0, "/opt/trn_rl_repo")

import numpy as np
import ml_dtypes

import concourse.bass as bass
import concourse.tile as tile
from concourse import bacc, mybir
from concourse.bass_utils import run_bass_kernel_spmd

BF16 = ml_dtypes.bfloat16

# Problem sizes (hardcoded per harness contract).
N_NODES = 50000
N_EDGES = 800000
D_X, D_E, D_U = 128, 96, 64
HID, D_OUT = 256, 128
BN_EPS = 1e-5

N_CORES = 8
NPC = N_NODES // N_CORES          # nodes per core = 6250
BLK = 128                         # nodes per scatter block
NBLK = (NPC + BLK - 1) // BLK     # blocks per core = 49
ST = 256                          # nodes per MLP supertile
NST = (NPC + ST - 1) // ST        # supertiles per core = 25

F32 = mybir.dt.float32
F32R = mybir.dt.float32r
BF = mybir.dt.bfloat16

# Fraction of value-hot builds routed to GpSimd to unload the DVE.
VHOT_GPS_EVERY = 4  # every 4th tile -> gpsimd; 0 disables

USE_F32R = False


def _r(ap):
    """fp32 -> float32r view for the PE fast path."""
    return ap.bitcast(F32R) if USE_F32R else ap


def build_device_kernel(S, T):
    """Build the single SPMD program. S = edge tiles per block, T = NBLK*S."""
    nc = bacc.Bacc("TRN2", target_bir_lowering=False, debug=False,
                   num_devices=N_CORES)

    # Per-core inputs.
    edges = nc.dram_tensor("edges", [128, T * D_E], BF, kind="ExternalInput")
    rels = nc.dram_tensor("rels", [128, T], F32, kind="ExternalInput")
    vals = nc.dram_tensor("vals", [128, T], F32, kind="ExternalInput")
    actx = nc.dram_tensor("actx", [D_X, NPC], F32, kind="ExternalInput")
    actu = nc.dram_tensor("actu", [D_U, NPC], F32, kind="ExternalInput")
    # Replicated inputs.
    w1x = nc.dram_tensor("w1x", [D_X, HID], F32, kind="ExternalInput")
    w1u = nc.dram_tensor("w1u", [D_U, HID], F32, kind="ExternalInput")
    w1e = nc.dram_tensor("w1e", [D_E, HID], F32, kind="ExternalInput")
    w2 = nc.dram_tensor("w2", [HID, D_OUT], F32, kind="ExternalInput")
    b2t = nc.dram_tensor("b2t", [1, D_OUT], F32, kind="ExternalInput")
    gam = nc.dram_tensor("gam", [128, 2], F32, kind="ExternalInput")
    bet = nc.dram_tensor("bet", [128, 2], F32, kind="ExternalInput")
    iot = nc.dram_tensor("iot", [128, BLK], BF, kind="ExternalInput")
    ones = nc.dram_tensor("ones", [1, ST], F32, kind="ExternalInput")

    outT = nc.dram_tensor("outT", [D_OUT, NPC], F32, kind="ExternalOutput")

    with tile.TileContext(nc) as tc:
        with (
            tc.tile_pool(name="const", bufs=1) as cpool,
            tc.tile_pool(name="big", bufs=1) as bigpool,
            tc.tile_pool(name="edges", bufs=3) as epool,
            tc.tile_pool(name="vhot", bufs=6) as vpool,
            tc.tile_pool(name="stage", bufs=3) as spool,
            tc.tile_pool(name="small", bufs=2) as smpool,
            tc.tile_pool(name="pse", bufs=2, space="PSUM") as pse,
            tc.tile_pool(name="psm1", bufs=3, space="PSUM") as psm1,
            tc.tile_pool(name="psm2", bufs=2, space="PSUM") as psm2,
            tc.tile_pool(name="dram", bufs=2, space="DRAM") as dpool,
        ):
            # ---- load constants / persistent buffers ----
            iota_sb = cpool.tile([128, BLK], BF)
            nc.sync.dma_start(iota_sb[:], iot.ap())
            rel_sb = cpool.tile([128, T], F32)
            nc.sync.dma_start(rel_sb[:], rels.ap())
            val_sb = cpool.tile([128, T], F32)
            nc.sync.dma_start(val_sb[:], vals.ap())
            w1x_sb = cpool.tile([D_X, HID], F32)
            nc.sync.dma_start(w1x_sb[:], w1x.ap())
            w1u_sb = cpool.tile([D_U, HID], F32)
            nc.sync.dma_start(w1u_sb[:], w1u.ap())
            w1e_sb = cpool.tile([D_E, HID], F32)
            nc.sync.dma_start(w1e_sb[:], w1e.ap())
            w2_sb = cpool.tile([HID // 2, 2 * D_OUT], F32)  # [128, 2*128]: chunk j in cols j*128..
            nc.sync.dma_start(w2_sb[:, 0:D_OUT], w2.ap()[0:128, :])
            nc.sync.dma_start(w2_sb[:, D_OUT:2 * D_OUT], w2.ap()[128:256, :])
            b2_sb = cpool.tile([1, D_OUT], F32)
            nc.sync.dma_start(b2_sb[:], b2t.ap())
            ones_sb = cpool.tile([1, ST], F32)
            nc.sync.dma_start(ones_sb[:], ones.ap())
            eps_sb = cpool.tile([128, 1], F32)
            nc.gpsimd.memset(eps_sb[:], BN_EPS)
            gam_sb = cpool.tile([128, 2], F32)
            nc.sync.dma_start(gam_sb[:], gam.ap())
            bet_sb = cpool.tile([128, 2], F32)
            nc.sync.dma_start(bet_sb[:], bet.ap())

            actx_sb = bigpool.tile([D_X, NPC], F32)
            nc.sync.dma_start(actx_sb[:], actx.ap())
            actu_sb = bigpool.tile([D_U, NPC], F32)
            nc.sync.dma_start(actu_sb[:], actu.ap())

            eagg = bigpool.tile([D_E, NBLK * BLK], F32)
            pt0 = bigpool.tile([128, NPC], F32, tag="pt0")
            pt1 = bigpool.tile([128, NPC], F32, tag="pt1")
            pt = [pt0, pt1]
            hrl0 = bigpool.tile([128, NPC], F32, tag="hr0")
            hrl1 = bigpool.tile([128, NPC], F32, tag="hr1")
            hrl = [hrl0, hrl1]
            stats = smpool.tile([128, 4], F32)

            # ---- Phase A: segment-mean into eagg (feature-major) ----
            for b in range(NBLK):
                et = epool.tile([128, S * D_E], BF)
                nc.sync.dma_start(et[:], edges.ap()[:, b * S * D_E:(b + 1) * S * D_E])
                ps = pse.tile([D_E, BLK], F32, space="PSUM")
                for s in range(S):
                    t = b * S + s
                    vh = vpool.tile([128, BLK], BF, tag="vh")
                    eng = (nc.gpsimd if (VHOT_GPS_EVERY and s % VHOT_GPS_EVERY == VHOT_GPS_EVERY - 1)
                           else nc.vector)
                    eng.tensor_scalar(
                        out=vh[:], in0=iota_sb[:],
                        scalar1=rel_sb[:, t:t + 1], scalar2=val_sb[:, t:t + 1],
                        op0=mybir.AluOpType.is_equal, op1=mybir.AluOpType.mult,
                    )
                    nc.tensor.matmul(
                        out=ps[:], lhsT=et[:, s * D_E:(s + 1) * D_E], rhs=vh[:],
                        start=(s == 0), stop=(s == S - 1),
                    )
                nc.scalar.copy(eagg[:, b * BLK:(b + 1) * BLK], ps[:])

            # ---- Phase B: MLP1, feature-major, fp32(r) ----
            for st in range(NST):
                n0 = st * ST
                nn = min(ST, NPC - n0)
                for h in range(2):
                    ps = psm1.tile([128, ST], F32, space="PSUM", tag="m1")
                    hs = slice(h * 128, (h + 1) * 128)
                    nc.tensor.matmul(out=ps[:, :nn], lhsT=_r(w1x_sb[:, hs]),
                                     rhs=_r(actx_sb[:, n0:n0 + nn]),
                                     start=True, stop=False)
                    nc.tensor.matmul(out=ps[:, :nn], lhsT=_r(w1u_sb[:, hs]),
                                     rhs=_r(actu_sb[:, n0:n0 + nn]),
                                     start=False, stop=False)
                    nc.tensor.matmul(out=ps[:, :nn], lhsT=_r(w1e_sb[:, hs]),
                                     rhs=_r(eagg[:, n0:n0 + nn]),
                                     start=False, stop=True)
                    nc.scalar.copy(pt[h][:, n0:n0 + nn], ps[:, :nn])

            # ---- Phase C: BN statistics + AllReduce ----
            for h in range(2):
                nc.vector.tensor_reduce(out=stats[:, h:h + 1], in_=pt[h][:],
                                        axis=mybir.AxisListType.X,
                                        op=mybir.AluOpType.add)
                # Square into the (not-yet-used) hrelu buffer; only the
                # per-partition accumulated sum is kept.
                nc.scalar.activation(out=hrl[h][:], in_=pt[h][:],
                                     func=mybir.ActivationFunctionType.Square,
                                     accum_out=stats[:, 2 + h:3 + h])

            cc_in = dpool.tile([128, 4], F32)
            cc_out = dpool.tile([128, 4], F32)
            nc.gpsimd.dma_start(cc_in[:], stats[:])
            nc.gpsimd.collective_compute(
                "AllReduce", mybir.AluOpType.add,
                replica_groups=[list(range(N_CORES))],
                ins=[cc_in.opt()], outs=[cc_out.opt()],
            )
            gstats = smpool.tile([128, 4], F32)
            nc.gpsimd.dma_start(gstats[:], cc_out[:])

            mean = smpool.tile([128, 2], F32)
            nc.vector.tensor_scalar_mul(out=mean[:], in0=gstats[:, 0:2],
                                        scalar1=1.0 / N_NODES)
            var = smpool.tile([128, 2], F32)
            nc.vector.tensor_scalar_mul(out=var[:], in0=gstats[:, 2:4],
                                        scalar1=1.0 / N_NODES)
            msq = smpool.tile([128, 2], F32)
            nc.vector.tensor_tensor(out=msq[:], in0=mean[:], in1=mean[:],
                                    op=mybir.AluOpType.mult)
            nc.vector.tensor_tensor(out=var[:], in0=var[:], in1=msq[:],
                                    op=mybir.AluOpType.subtract)
            std = smpool.tile([128, 2], F32)
            nc.scalar.activation(out=std[:], in_=var[:],
                                 func=mybir.ActivationFunctionType.Sqrt,
                                 bias=eps_sb[:, 0:1])
            rstd = smpool.tile([128, 2], F32)
            nc.vector.reciprocal(out=rstd[:], in_=std[:])
            scale = smpool.tile([128, 2], F32)
            nc.vector.tensor_tensor(out=scale[:], in0=rstd[:], in1=gam_sb[:],
                                    op=mybir.AluOpType.mult)
            shift = smpool.tile([128, 2], F32)
            nc.vector.tensor_tensor(out=shift[:], in0=mean[:], in1=scale[:],
                                    op=mybir.AluOpType.mult)
            nc.vector.tensor_tensor(out=shift[:], in0=bet_sb[:], in1=shift[:],
                                    op=mybir.AluOpType.subtract)

            # ---- Phase D: fused BN + ReLU ----
            for h in range(2):
                nc.scalar.activation(out=hrl[h][:], in_=pt[h][:],
                                     func=mybir.ActivationFunctionType.Relu,
                                     scale=scale[:, h:h + 1],
                                     bias=shift[:, h:h + 1])

            # ---- Phase E: MLP2 (+b2), output transposed ----
            for st in range(NST):
                n0 = st * ST
                nn = min(ST, NPC - n0)
                ps = psm2.tile([128, ST], F32, space="PSUM", tag="m2")
                nc.tensor.matmul(out=ps[:, :nn], lhsT=_r(w2_sb[:, 0:D_OUT]),
                                 rhs=_r(hrl[0][:, n0:n0 + nn]),
                                 start=True, stop=False)
                nc.tensor.matmul(out=ps[:, :nn], lhsT=_r(w2_sb[:, D_OUT:2 * D_OUT]),
                                 rhs=_r(hrl[1][:, n0:n0 + nn]),
                                 start=False, stop=False)
                nc.tensor.matmul(out=ps[:, :nn], lhsT=_r(b2_sb[:]),
                                 rhs=_r(ones_sb[:, :nn]),
                                 start=False, stop=True)
                sg = spool.tile([D_OUT, ST], F32, tag="og")
                nc.scalar.copy(sg[:, :nn], ps[:, :nn])
                nc.sync.dma_start(outT.ap()[:, n0:n0 + nn], sg[:, :nn])

    nc.compile()
    return nc


def host_prep(x, edge_index, edge_attr, u, batch, W1, b1, gamma, beta, W2, b2):
    """Shard + stage inputs. Index bookkeeping only (sort/bin/layout); the
    arithmetic of the model itself runs on device."""
    x = np.asarray(x, np.float32)
    edge_attr = np.asarray(edge_attr, np.float32)
    u = np.asarray(u, np.float32)
    W1 = np.asarray(W1, np.float32)
    W2 = np.asarray(W2, np.float32)
    b2 = np.asarray(b2, np.float32)
    gamma = np.asarray(gamma, np.float32)
    beta = np.asarray(beta, np.float32)
    col = np.asarray(edge_index)[1].astype(np.int64)
    batch = np.asarray(batch).astype(np.int64)

    order = np.argsort(col, kind="stable")
    col_s = col[order]

    cnt = np.bincount(col, minlength=N_NODES).astype(np.int64)
    inv = np.zeros(N_NODES, np.float32)
    nz = cnt > 0
    inv[nz] = 1.0 / cnt[nz]

    core = col_s // NPC
    blk = (col_s % NPC) // BLK
    g = core * NBLK + blk
    gcnt = np.bincount(g, minlength=N_CORES * NBLK)
    S = int(np.max((gcnt + 127) // 128))
    T = NBLK * S

    starts = np.zeros(N_CORES * NBLK, np.int64)
    np.cumsum(gcnt[:-1], out=starts[1:])
    rank = np.arange(N_EDGES, dtype=np.int64) - starts[g]
    t_in = rank // 128
    p = rank % 128
    tglob = blk * S + t_in

    # Edge payloads: [core][partition p][tile t][feature] layout, bf16.
    ebuf = np.zeros((N_CORES, 128, T, D_E), BF16)
    ebuf[core, p, tglob, :] = edge_attr[order].astype(BF16)
    rbuf = np.full((N_CORES, 128, T), 255.0, np.float32)
    rbuf[core, p, tglob] = ((col_s % NPC) % BLK).astype(np.float32)
    vbuf = np.zeros((N_CORES, 128, T), np.float32)
    vbuf[core, p, tglob] = inv[col_s]

    actx = np.ascontiguousarray(x.T)                     # [128, N]
    actu = np.ascontiguousarray(u[batch].T)              # [64, N]

    # W1 rows: 0:128 -> x, 128:224 -> e_aggr, 224:288 -> u (reference concat
    # order). b1 is intentionally dropped (BatchNorm removes it exactly).
    w1x = np.ascontiguousarray(W1[0:D_X])
    w1e = np.ascontiguousarray(W1[D_X:D_X + D_E])
    w1u = np.ascontiguousarray(W1[D_X + D_E:])

    gam = np.ascontiguousarray(gamma.reshape(2, 128).T)
    bet = np.ascontiguousarray(beta.reshape(2, 128).T)
    iot = np.broadcast_to(np.arange(BLK, dtype=np.float32).astype(BF16)[None, :],
                          (128, BLK)).copy()
    onesv = np.ones((1, ST), np.float32)
    b2t = b2.reshape(1, D_OUT)

    in_maps = []
    for c in range(N_CORES):
        in_maps.append({
            "edges": ebuf[c].reshape(128, T * D_E),
            "rels": rbuf[c],
            "vals": vbuf[c],
            "actx": np.ascontiguousarray(actx[:, c * NPC:(c + 1) * NPC]),
            "actu": np.ascontiguousarray(actu[:, c * NPC:(c + 1) * NPC]),
            "w1x": w1x, "w1u": w1u, "w1e": w1e,
            "w2": W2, "b2t": b2t, "gam": gam, "bet": bet,
            "iot": iot, "ones": onesv,
        })
    return in_maps, S, T


_CACHE = {}


def kernel(**inputs):
    in_maps, S, T = host_prep(**inputs)
    nc = _CACHE.get(S)
    if nc is None:
        nc = build_device_kernel(S, T)
        _CACHE[S] = nc
    res = run_bass_kernel_spmd(
        nc, in_maps, core_ids=list(range(N_CORES)),
        trace=bool(int(os.environ.get("KERNEL_TRACE", "0"))),
    )
    kernel.last_results = res
    out = np.concatenate(
        [res.results[c]["outT"].T for c in range(N_CORES)], axis=0)
    return np.ascontiguousarray(out.astype(np.float32))


# revision 10
# speedup vs baseline: 2.1172x; 2.1172x over previous
"""GNN NodeModel kernel for 8 Trainium2 NeuronCores.

Computation:
    e_aggr = scatter_mean(edge_attr, edge_index[1], n_nodes)       # [N, 96]
    h      = concat([x, e_aggr, u[batch]], 1) @ W1 + b1            # [N, 256]
    h      = BatchNorm1d(h, training batch stats); relu            # global stats
    out    = h @ W2 + b2                                           # [N, 128]

Sharding: nodes are split into 8 contiguous ranges of 6250 (one per core);
edges are binned by *target* node so every scatter target is core-local.
MLP weights are replicated; BatchNorm statistics are AllReduced on-device.

Scatter layout (the core trick): within each core the nodes are sorted by
degree (descending) and renumbered; 128 consecutive sorted nodes form a
block.  Edges live in an ELL layout [partition = node-in-block,
slot = edge-rank-within-node]; because same-block nodes have nearly equal
degrees, padding to the per-block slot count S_b (= max degree in that
block over all cores, so one SPMD program fits every core) wastes only a
few %.  The segment-mean for a block is then pure PE work:

    e_aggrT[96, 128] = sum_s  slab_s[128n, 96]^T @ diag(1/cnt)[128, 128]

i.e. each slot tile is transpose-accumulated into PSUM via a matmul whose
moving operand is a per-block diagonal matrix carrying 1/cnt — no
per-edge-tile vector-engine work at all (one tiny diag build per block).
Pad slots hold zeros; zero-degree nodes get diag value 0.

MLP1/MLP2 run feature-major (P^T = W^T @ act^T) so no transposes are ever
needed; operands are cast to float32r (the PE's 1-cycle/row fp32 path —
plain fp32 matmul costs 4 cycles/row).  b1 is dropped: BatchNorm is exactly
invariant to a per-feature constant shift, for any b1.  BN + ReLU fuse into
one scalar-engine activation per half.  The output is produced transposed;
the host transposes and un-permutes while unsharding.

Edge values ride in bf16 (their h-contribution is small and BatchNorm is
scale-invariant); everything downstream accumulates in fp32 PSUM.
"""

import os
import sys

sys.path.insert(0, "/opt/trn_rl_repo")

import numpy as np
import ml_dtypes

import concourse.bass as bass
import concourse.tile as tile
from concourse import bacc, mybir
from concourse.bass_utils import run_bass_kernel_spmd

BF16 = ml_dtypes.bfloat16

# Problem sizes (hardcoded per harness contract).
N_NODES = 50000
N_EDGES = 800000
D_X, D_E, D_U = 128, 96, 64
HID, D_OUT = 256, 128
BN_EPS = 1e-5

N_CORES = 8
NPC = N_NODES // N_CORES          # nodes per core = 6250
BLK = 128                         # nodes per scatter block
NBLK = (NPC + BLK - 1) // BLK     # blocks per core = 49
ST = 256                          # nodes per MLP supertile
NST = (NPC + ST - 1) // ST        # supertiles per core = 25

F32 = mybir.dt.float32
F32R = mybir.dt.float32r
BF = mybir.dt.bfloat16


def build_device_kernel(sb_list):
    """One SPMD program for all 8 cores.

    sb_list: per-block edge-slot counts S_b (len NBLK), identical across
    cores by construction.
    """
    S0 = max(sb_list)
    tile_base = np.concatenate([[0], np.cumsum(sb_list)]).astype(int)

    nc = bacc.Bacc("TRN2", target_bir_lowering=False, debug=False,
                   num_devices=N_CORES)

    TOT = int(tile_base[-1])
    # Per-core inputs.
    edges = nc.dram_tensor("edges", [128, TOT * D_E], BF, kind="ExternalInput")
    invs = nc.dram_tensor("invs", [128, NBLK], F32, kind="ExternalInput")
    actx = nc.dram_tensor("actx", [D_X, NPC], F32, kind="ExternalInput")
    actu = nc.dram_tensor("actu", [D_U, NPC], F32, kind="ExternalInput")
    # Replicated inputs.
    w1x = nc.dram_tensor("w1x", [D_X, HID], F32, kind="ExternalInput")
    w1u = nc.dram_tensor("w1u", [D_U, HID], F32, kind="ExternalInput")
    w1e = nc.dram_tensor("w1e", [D_E, HID], F32, kind="ExternalInput")
    w2 = nc.dram_tensor("w2", [HID, D_OUT], F32, kind="ExternalInput")
    b2t = nc.dram_tensor("b2t", [1, D_OUT], F32, kind="ExternalInput")
    gam = nc.dram_tensor("gam", [128, 2], F32, kind="ExternalInput")
    bet = nc.dram_tensor("bet", [128, 2], F32, kind="ExternalInput")
    idn = nc.dram_tensor("idn", [128, BLK], BF, kind="ExternalInput")
    ones = nc.dram_tensor("ones", [1, ST], F32, kind="ExternalInput")

    outT = nc.dram_tensor("outT", [D_OUT, NPC], F32, kind="ExternalOutput")

    with tile.TileContext(nc) as tc:
        with (
            tc.tile_pool(name="const", bufs=1) as cpool,
            tc.tile_pool(name="big", bufs=1) as bigpool,
            tc.tile_pool(name="edges", bufs=2) as epool,
            tc.tile_pool(name="diag", bufs=4) as dgpool,
            tc.tile_pool(name="stage", bufs=3) as spool,
            tc.tile_pool(name="cast", bufs=2) as capool,
            tc.tile_pool(name="small", bufs=2) as smpool,
            tc.tile_pool(name="pse", bufs=2, space="PSUM") as pse,
            tc.tile_pool(name="psm1", bufs=3, space="PSUM") as psm1,
            tc.tile_pool(name="psm2", bufs=2, space="PSUM") as psm2,
            tc.tile_pool(name="dram", bufs=2, space="DRAM") as dpool,
        ):
            # ---- constants / persistent buffers ----
            ident_sb = cpool.tile([128, BLK], BF)
            nc.sync.dma_start(ident_sb[:], idn.ap())
            inv_sb = cpool.tile([128, NBLK], F32)
            nc.sync.dma_start(inv_sb[:], invs.ap())
            gam_sb = cpool.tile([128, 2], F32)
            nc.sync.dma_start(gam_sb[:], gam.ap())
            bet_sb = cpool.tile([128, 2], F32)
            nc.sync.dma_start(bet_sb[:], bet.ap())
            eps_sb = cpool.tile([128, 1], F32)
            nc.gpsimd.memset(eps_sb[:], BN_EPS)

            # Weights: load fp32, cast once to float32r.
            def load_cast(name, src_ap, p, f):
                raw = capool.tile([p, f], F32, tag="wraw")
                nc.sync.dma_start(raw[:], src_ap)
                r = cpool.tile([p, f], F32R, tag=f"{name}_r")
                nc.scalar.copy(r[:], raw[:])
                return r

            w1x_sb = load_cast("w1x", w1x.ap(), D_X, HID)
            w1u_sb = load_cast("w1u", w1u.ap(), D_U, HID)
            w1e_sb = load_cast("w1e", w1e.ap(), D_E, HID)
            w2a_sb = load_cast("w2a", w2.ap()[0:128, :], 128, D_OUT)
            w2b_sb = load_cast("w2b", w2.ap()[128:256, :], 128, D_OUT)
            b2_sb = load_cast("b2", b2t.ap(), 1, D_OUT)
            ones_sb = load_cast("ones", ones.ap(), 1, ST)

            # Activations: stream in fp32 chunks, cast to persistent f32r.
            actxr = bigpool.tile([D_X, NPC], F32R)
            actur = bigpool.tile([D_U, NPC], F32R)
            CCH = 625
            for i in range(NPC // CCH):
                cs = slice(i * CCH, (i + 1) * CCH)
                cx = capool.tile([D_X, CCH], F32, tag="cx")
                nc.sync.dma_start(cx[:], actx.ap()[:, cs])
                nc.scalar.copy(actxr[:, cs], cx[:])
                cu = capool.tile([D_U, CCH], F32, tag="cu")
                nc.sync.dma_start(cu[:], actu.ap()[:, cs])
                nc.scalar.copy(actur[:, cs], cu[:])

            eagg = bigpool.tile([D_E, NBLK * BLK], F32R)
            pt0 = bigpool.tile([128, NPC], F32, tag="pt0")
            pt1 = bigpool.tile([128, NPC], F32, tag="pt1")
            pt = [pt0, pt1]
            hrl0 = bigpool.tile([128, NPC], F32R, tag="hr0")
            hrl1 = bigpool.tile([128, NPC], F32R, tag="hr1")
            hrl = [hrl0, hrl1]
            stats = smpool.tile([128, 4], F32)

            # ---- Phase A: segment-mean into eagg (feature-major) ----
            for b in range(NBLK):
                sb = sb_list[b]
                et = epool.tile([128, S0 * D_E], BF, tag="slab")
                nc.sync.dma_start(
                    et[:, :sb * D_E],
                    edges.ap()[:, tile_base[b] * D_E:(tile_base[b] + sb) * D_E])
                dg = dgpool.tile([128, BLK], BF, tag="dg")
                nc.vector.tensor_scalar_mul(out=dg[:], in0=ident_sb[:],
                                            scalar1=inv_sb[:, b:b + 1])
                ps = pse.tile([D_E, BLK], F32, space="PSUM")
                for s in range(sb):
                    nc.tensor.matmul(
                        out=ps[:], lhsT=et[:, s * D_E:(s + 1) * D_E], rhs=dg[:],
                        start=(s == 0), stop=(s == sb - 1),
                    )
                nc.scalar.copy(eagg[:, b * BLK:(b + 1) * BLK], ps[:])

            # ---- Phase B: MLP1, feature-major, float32r ----
            for st in range(NST):
                n0 = st * ST
                nn = min(ST, NPC - n0)
                for h in range(2):
                    ps = psm1.tile([128, ST], F32, space="PSUM", tag="m1")
                    hs = slice(h * 128, (h + 1) * 128)
                    nc.tensor.matmul(out=ps[:, :nn], lhsT=w1x_sb[:, hs],
                                     rhs=actxr[:, n0:n0 + nn],
                                     start=True, stop=False)
                    nc.tensor.matmul(out=ps[:, :nn], lhsT=w1u_sb[:, hs],
                                     rhs=actur[:, n0:n0 + nn],
                                     start=False, stop=False)
                    nc.tensor.matmul(out=ps[:, :nn], lhsT=w1e_sb[:, hs],
                                     rhs=eagg[:, n0:n0 + nn],
                                     start=False, stop=True)
                    nc.scalar.copy(pt[h][:, n0:n0 + nn], ps[:, :nn])

            # ---- Phase C: BN statistics + AllReduce ----
            for h in range(2):
                nc.vector.tensor_reduce(out=stats[:, h:h + 1], in_=pt[h][:],
                                        axis=mybir.AxisListType.X,
                                        op=mybir.AluOpType.add)
                # Square into the not-yet-used hrelu buffer; keep only the
                # per-partition accumulated sum.
                nc.scalar.activation(out=hrl[h][:], in_=pt[h][:],
                                     func=mybir.ActivationFunctionType.Square,
                                     accum_out=stats[:, 2 + h:3 + h])

            cc_in = dpool.tile([128, 4], F32)
            cc_out = dpool.tile([128, 4], F32)
            nc.gpsimd.dma_start(cc_in[:], stats[:])
            nc.gpsimd.collective_compute(
                "AllReduce", mybir.AluOpType.add,
                replica_groups=[list(range(N_CORES))],
                ins=[cc_in.opt()], outs=[cc_out.opt()],
            )
            gstats = smpool.tile([128, 4], F32)
            nc.gpsimd.dma_start(gstats[:], cc_out[:])

            mean = smpool.tile([128, 2], F32)
            nc.vector.tensor_scalar_mul(out=mean[:], in0=gstats[:, 0:2],
                                        scalar1=1.0 / N_NODES)
            var = smpool.tile([128, 2], F32)
            nc.vector.tensor_scalar_mul(out=var[:], in0=gstats[:, 2:4],
                                        scalar1=1.0 / N_NODES)
            msq = smpool.tile([128, 2], F32)
            nc.vector.tensor_tensor(out=msq[:], in0=mean[:], in1=mean[:],
                                    op=mybir.AluOpType.mult)
            nc.vector.tensor_tensor(out=var[:], in0=var[:], in1=msq[:],
                                    op=mybir.AluOpType.subtract)
            std = smpool.tile([128, 2], F32)
            nc.scalar.activation(out=std[:], in_=var[:],
                                 func=mybir.ActivationFunctionType.Sqrt,
                                 bias=eps_sb[:, 0:1])
            rstd = smpool.tile([128, 2], F32)
            nc.vector.reciprocal(out=rstd[:], in_=std[:])
            scale = smpool.tile([128, 2], F32)
            nc.vector.tensor_tensor(out=scale[:], in0=rstd[:], in1=gam_sb[:],
                                    op=mybir.AluOpType.mult)
            shift = smpool.tile([128, 2], F32)
            nc.vector.tensor_tensor(out=shift[:], in0=mean[:], in1=scale[:],
                                    op=mybir.AluOpType.mult)
            nc.vector.tensor_tensor(out=shift[:], in0=bet_sb[:], in1=shift[:],
                                    op=mybir.AluOpType.subtract)

            # ---- Phase D: fused BN + ReLU (writes float32r) ----
            for h in range(2):
                nc.scalar.activation(out=hrl[h][:], in_=pt[h][:],
                                     func=mybir.ActivationFunctionType.Relu,
                                     scale=scale[:, h:h + 1],
                                     bias=shift[:, h:h + 1])

            # ---- Phase E: MLP2 (+b2), output transposed ----
            for st in range(NST):
                n0 = st * ST
                nn = min(ST, NPC - n0)
                ps = psm2.tile([128, ST], F32, space="PSUM", tag="m2")
                nc.tensor.matmul(out=ps[:, :nn], lhsT=w2a_sb[:],
                                 rhs=hrl[0][:, n0:n0 + nn],
                                 start=True, stop=False)
                nc.tensor.matmul(out=ps[:, :nn], lhsT=w2b_sb[:],
                                 rhs=hrl[1][:, n0:n0 + nn],
                                 start=False, stop=False)
                nc.tensor.matmul(out=ps[:, :nn], lhsT=b2_sb[:],
                                 rhs=ones_sb[:, :nn],
                                 start=False, stop=True)
                sg = spool.tile([D_OUT, ST], F32, tag="og")
                nc.scalar.copy(sg[:, :nn], ps[:, :nn])
                nc.sync.dma_start(outT.ap()[:, n0:n0 + nn], sg[:, :nn])

    nc.compile()
    return nc


def host_prep(x, edge_index, edge_attr, u, batch, W1, b1, gamma, beta, W2, b2):
    """Shard + stage inputs. Index bookkeeping only (sort/bin/layout); the
    model arithmetic itself runs on device."""
    x = np.asarray(x, np.float32)
    edge_attr = np.asarray(edge_attr, np.float32)
    u = np.asarray(u, np.float32)
    W1 = np.asarray(W1, np.float32)
    W2 = np.asarray(W2, np.float32)
    b2 = np.asarray(b2, np.float32)
    gamma = np.asarray(gamma, np.float32)
    beta = np.asarray(beta, np.float32)
    col = np.asarray(edge_index)[1].astype(np.int64)
    batch = np.asarray(batch).astype(np.int64)

    deg = np.bincount(col, minlength=N_NODES).astype(np.int64)
    inv = np.zeros(N_NODES, np.float32)
    nz = deg > 0
    inv[nz] = 1.0 / deg[nz]

    # Per-core degree-descending node order (stable).
    deg2 = deg.reshape(N_CORES, NPC)
    perm = np.argsort(-deg2, axis=1, kind="stable")   # [8, NPC]: orig local id by rank
    rank_of = np.empty_like(perm)
    np.put_along_axis(
        rank_of, perm, np.broadcast_to(np.arange(NPC)[None, :], perm.shape).copy(),
        axis=1)

    # Per-block slot counts: the block's max degree (first element in desc
    # order), maxed over cores so one program serves all cores.
    deg_sorted = np.take_along_axis(deg2, perm, axis=1)
    sb_list = [max(1, int(deg_sorted[:, b * 128].max())) for b in range(NBLK)]
    tile_base = np.concatenate([[0], np.cumsum(sb_list)]).astype(np.int64)
    TOT = int(tile_base[-1])

    # Edge placement: target node -> (core, rank) -> (block, partition, slot).
    ecore = col // NPC
    local = col % NPC
    erank = rank_of[ecore, local]
    eb = erank // 128
    ep = erank % 128
    order = np.argsort(col, kind="stable")
    sidx = np.empty(N_EDGES, np.int64)
    starts = np.zeros(N_NODES + 1, np.int64)
    np.cumsum(deg, out=starts[1:])
    sidx[order] = np.arange(N_EDGES) - starts[col[order]]  # edge rank within node

    ebuf = np.zeros((N_CORES, 128, TOT, D_E), BF16)
    ebuf[ecore, ep, tile_base[eb] + sidx, :] = edge_attr.astype(BF16)

    invb = np.zeros((N_CORES, 128, NBLK), np.float32)
    rr = np.arange(NPC)
    invb[:, rr % 128, rr // 128] = np.take_along_axis(
        inv.reshape(N_CORES, NPC), perm, axis=1)

    # Feature-major activations in sorted-node order.
    gperm = (perm + (np.arange(N_CORES) * NPC)[:, None]).reshape(-1)
    actx = np.ascontiguousarray(x[gperm].T)                # [128, N]
    actu = np.ascontiguousarray(u[batch[gperm]].T)         # [64, N]

    # W1 rows follow the reference concat order [x | e_aggr | u].
    w1x = np.ascontiguousarray(W1[0:D_X])
    w1e = np.ascontiguousarray(W1[D_X:D_X + D_E])
    w1u = np.ascontiguousarray(W1[D_X + D_E:])

    gam = np.ascontiguousarray(gamma.reshape(2, 128).T)
    bet = np.ascontiguousarray(beta.reshape(2, 128).T)
    idn = np.eye(128, dtype=np.float32).astype(BF16)
    onesv = np.ones((1, ST), np.float32)
    b2t = b2.reshape(1, D_OUT)

    in_maps = []
    for c in range(N_CORES):
        in_maps.append({
            "edges": ebuf[c].reshape(128, TOT * D_E),
            "invs": invb[c],
            "actx": np.ascontiguousarray(actx[:, c * NPC:(c + 1) * NPC]),
            "actu": np.ascontiguousarray(actu[:, c * NPC:(c + 1) * NPC]),
            "w1x": w1x, "w1u": w1u, "w1e": w1e,
            "w2": W2, "b2t": b2t, "gam": gam, "bet": bet,
            "idn": idn, "ones": onesv,
        })
    return in_maps, tuple(sb_list), gperm


_CACHE = {}


def kernel(**inputs):
    in_maps, sb_list, gperm = host_prep(**inputs)
    nc = _CACHE.get(sb_list)
    if nc is None:
        nc = build_device_kernel(list(sb_list))
        _CACHE[sb_list] = nc
    res = run_bass_kernel_spmd(
        nc, in_maps, core_ids=list(range(N_CORES)),
        trace=bool(int(os.environ.get("KERNEL_TRACE", "0"))),
    )
    kernel.last_results = res
    outp = np.concatenate(
        [res.results[c]["outT"].T for c in range(N_CORES)], axis=0)
    out = np.empty((N_NODES, D_OUT), np.float32)
    out[gperm] = outp
    return np.ascontiguousarray(out)
